# revision 48
# baseline (speedup 1.0000x reference)
"""DistMaps Trainium2 kernel (separable low-rank matmul formulation).

out[g, r, c] = tanh(2*sqrt(min_j d2_j(r, c))) with d2_j = ((r-pr_j)^2 +
(c-pc_j)^2)/25.  The per-click "dip" D(dr, dc) = 1 - tanh(0.4*sqrt(dr^2 +
dc^2)) has compact support (|dr|,|dc| <= ~47 px, beyond which tanh
saturates to 1.0 in fp32).  Approximate D as a rank-M separable expansion
D ~= sum_m f_m(|dr|) g_m(|dc|) (SVD of the radial bump, hardcoded basis)
and compute, per 128-row block,

    out_block = 1 - sum_clicks sum_m f_m g_m

as a SINGLE K<=80 matmul into PSUM: lhsT rows hold [ones; -f_m sampled on
the 512 image rows], rhs holds [ones; g_m sampled on the 512 cols].  The
constant (ones x ones) K-row supplies the "+1", so PSUM holds the final
map directly.  Sum-over-clicks equals min-over-clicks up to the tiny
region where two dips overlap; clicks closer than 16 px are merged
host-side into one component whose exact joint dip gets its own SVD
rows.  A Relu pass (ScalarE / DVE alternating) copies PSUM->SBUF and
clamps the near-duplicate overshoot; per-block DMAs write the output.

The program is input-independent (compiled once, shared by all 8 cores);
per-core per-group tables [K, 1024] fp16 (lhsT | rhs) are the only
inputs.  8 cores run data-parallel over batch, one full [2, H, W] map
each.
"""

import sys

for _p in ("/opt/trn_rl_repo", "/root/.axon_site/_ro/trn_rl_repo"):
    if _p not in sys.path:
        sys.path.append(_p)

import numpy as np

import concourse.bass as bass
from concourse import bacc
import concourse.mybir as mybir
from concourse.tile import TileContext

B, C, H, W = 8, 3, 512, 512
P2 = 48
PG = 24
NCORES = 8
M = 3            # rank of the shared separable basis per click
K = 80           # fixed contraction depth (1 const row + click rows + spare)
DMERGE = 16.0    # clicks closer than this merge into an exact component
RCUT = 47.0      # dip support radius in px
STEP = 0.25      # basis sampling grid step

FP32 = mybir.dt.float32
FP16 = mybir.dt.float16


# ---------------------------------------------------------------------------
# Host-side tables
# ---------------------------------------------------------------------------

def _build_basis():
    s = np.arange(0.0, RCUT + STEP, STEP)
    G = 1.0 - np.tanh(0.4 * np.sqrt(s[:, None] ** 2 + s[None, :] ** 2))
    U, S, Vt = np.linalg.svd(G)
    f = U[:, :M] * np.sqrt(S[:M])[None, :]
    g = Vt[:M].T * np.sqrt(S[:M])[None, :]
    return s, f, g


_SGRID, _FBAS, _GBAS = _build_basis()
_COORD = np.arange(512, dtype=np.float64)


def _eval_basis(fb, offs):
    """Linear-interp the basis columns at |offs|; zero beyond support."""
    a = np.abs(offs)
    idx = np.clip(a / STEP, 0, len(_SGRID) - 1 - 1e-9)
    i0 = np.floor(idx).astype(np.int64)
    w = (idx - i0)[:, None]
    out = fb[i0] * (1 - w) + fb[i0 + 1] * w
    out[a > RCUT - STEP] = 0.0
    return out  # [512, M]


def _components(pts):
    """Union clicks closer than DMERGE. Returns list of index lists."""
    n = len(pts)
    parent = list(range(n))

    def find(i):
        while parent[i] != i:
            parent[i] = parent[parent[i]]
            i = parent[i]
        return i

    for i in range(n):
        for j in range(i + 1, n):
            if np.hypot(*(pts[i] - pts[j])) < DMERGE:
                parent[find(i)] = find(j)
    comps = {}
    for i in range(n):
        comps.setdefault(find(i), []).append(i)
    return list(comps.values())


def _group_table(pts_g):
    """[K, 1024] fp16 table for one group: [:, :512]=lhsT rows (over image
    rows), [:, 512:]=rhs rows (over image cols)."""
    tab = np.zeros((K, 1024), dtype=np.float64)
    tab[0, :] = 1.0  # const row: +1 into every pixel
    k = 1
    valid = pts_g[pts_g.max(axis=1) >= 0]
    for comp in _components(valid):
        if k >= K:
            break
        cp = valid[comp]
        if len(comp) == 1:
            pr, pc = cp[0]
            fr = _eval_basis(_FBAS, _COORD - pr)   # [512, M]
            gc = _eval_basis(_GBAS, _COORD - pc)
            take = min(M, K - k)
            tab[k:k + take, :512] = -fr[:, :take].T
            tab[k:k + take, 512:] = gc[:, :take].T
            k += take
        else:
            r0 = max(0, int(np.floor(cp[:, 0].min() - RCUT)))
            r1 = min(512, int(np.ceil(cp[:, 0].max() + RCUT)) + 1)
            c0 = max(0, int(np.floor(cp[:, 1].min() - RCUT)))
            c1 = min(512, int(np.ceil(cp[:, 1].max() + RCUT)) + 1)
            rr = _COORD[r0:r1]
            cc = _COORD[c0:c1]
            d2 = np.min(
                (rr[None, :, None] - cp[:, 0][:, None, None]) ** 2
                + (cc[None, None, :] - cp[:, 1][:, None, None]) ** 2,
                axis=0,
            )
            J = 1.0 - np.tanh(0.4 * np.sqrt(d2))
            U, S, Vt = np.linalg.svd(J, full_matrices=False)
            want = int((S >= 5e-3).sum())  # ranks until resid small
            take = min(max(want, 2 * len(comp)), M * len(comp) + 4, K - k)
            sq = np.sqrt(S[:take])
            tab[k:k + take, r0:r1] = -(U[:, :take] * sq).T
            tab[k:k + take, 512 + c0:512 + c1] = Vt[:take] * sq[:, None]
            k += take
    return tab.astype(np.float16)


def build_tables(coords_b):
    pts = coords_b[:, :2].astype(np.float64)
    return [_group_table(pts[g * PG:(g + 1) * PG]) for g in range(2)]


# ---------------------------------------------------------------------------
# Device program (input-independent; one program shared by all cores)
# ---------------------------------------------------------------------------

SWDGE_BLOCKS = (1, 3, 6, 7)  # blocks whose out-DMA goes via prepared SWDGE


def _fix_swdge_sync(nc, trigger_waits):
    """Post-finalize repairs for prepared (gen_mode=1) SWDGE DMAs, which
    Tile's pass-2 under-synchronizes in this snapshot:

    1. Consumers wait on the prep's DMASW<lane> semaphore, but the lane
       increment is never attached — the completion SyncUpdate (on_update[0],
       baked into the descriptor) still points at the caller's sem= handle.
       Repoint it at the lane semaphore so both the cost model's trigger
       drain and the hardware SDMA bump the sem consumers actually wait on.
    2. The deferred RAW edge (src producer -> trigger) is dropped from the
       trigger's semwaits; append an explicit engine-tick wait per
       (trigger, producer) pair recorded by the builder.
    """
    from concourse.tile_sem_assignment import PROC_NAME_TO_IDX

    idx_to_name = {v: k for k, v in PROC_NAME_TO_IDX.items()}
    fn = nc.m.functions[0]
    insts = [i for blk in fn.blocks for i in blk.instructions]

    lane_sem = {}
    for inst in insts:
        si = inst.sync_info
        if si is None:
            continue
        for w in si.on_wait or []:
            if w.ant_name and w.ant_name.startswith("DMASW"):
                lane_sem[w.ant_name.split("_")[0]] = w.id
    for inst in insts:
        if getattr(inst, "gen_mode", 0) != 1:
            continue
        name = idx_to_name.get(inst.bass_scheduled_proc)
        assert name in lane_sem, (name, lane_sem)
        inst.sync_info.on_update[0].id = lane_sem[name]

    # producer waits ride on the wait_ge guard preceding each trigger (an
    # event instruction, which can carry several waits; the trigger itself
    # and the relus are at their codegen sync-command limits)
    for guard, producers in trigger_waits:
        extra = []
        for prod in producers:
            tick = prod.bass_scheduled_tick
            proc = idx_to_name[prod.bass_scheduled_proc]
            upd = next(
                u for u in prod.sync_info.on_update
                if u.ant_name and u.ant_name.startswith(proc)
            )
            extra.append(
                mybir.SyncWait(
                    sync_type="semaphore", id=upd.id, ant_name=upd.ant_name,
                    wait_mode="sem-ge-imm", wait_value=tick,
                )
            )
        guard.sync_info.on_wait = extra


def build_program(swdge_blocks=SWDGE_BLOCKS, swdge_in=True, split0=True):
    nc = bacc.Bacc("TRN2", num_devices=1, debug=False, num_swdge_queues=4)
    tabs = [
        nc.dram_tensor(f"tab{g}", [K, 1024], FP16, kind="ExternalInput")
        for g in range(2)
    ]
    out = nc.dram_tensor("out", [2, H, W], FP32, kind="ExternalOutput")

    trigger_waits = []
    with TileContext(nc) as tc:
        with (
            tc.tile_pool(name="tabp", bufs=1) as tabp,
            tc.tile_pool(name="resp", bufs=8) as resp,
            tc.tile_pool(name="psp", bufs=6 if split0 else 8, space="PSUM") as psp,
            tc.tile_pool(name="hpsp", bufs=1, space="PSUM") as hpsp,
        ):
            # zero ctx-index tile for kv_writeback block writes
            zidx = tabp.tile([128, 1], mybir.dt.int32, tag="zidx")
            nc.gpsimd.memset(zidx[:], 0)
            if swdge_in:
                # wrapped gather indices: idx[p, j] = p + 16*j  (0..127)
                idxs = tabp.tile([16, 8], mybir.dt.int16, tag="idxs")
                nc.gpsimd.iota(idxs[:], [[16, 8]], base=0, channel_multiplier=1)

            # placeholder sem for the trigger guards (waits are rewritten to
            # the relu engine-tick sems post-finalize)
            guard_sem = nc.alloc_semaphore("guard_dummy")
            relu_sems = {}

            tt = [
                tabp.tile([128, 1024], FP16, tag=f"tab{g}", name=f"tab{g}")
                for g in range(2)
            ]
            # group-0 tables via prepared SWDGE gather (queue 0): descriptor
            # gen runs before the transfer window; trigger fires immediately
            if swdge_in:
                sem_in = nc.alloc_semaphore("in0_dma")
                nc.gpsimd.dma_gather(
                    tt[0].rearrange("p (j e) -> p j e", j=1),
                    tabs[0][:, :],
                    idxs[:, :5],
                    K,
                    K,
                    1024,
                    prepare_only=True,
                    sem=sem_in,
                    queue_num=0,
                )
                nc.gpsimd.trigger_dma(count=None, queue_num=0)
            else:
                nc.sync.dma_start(tt[0][:K, :], tabs[0][:, :])
            # group-1 tables via HWDGE on the idle SP queue (needed ~2us later)
            nc.sync.dma_start(tt[1][:K, :], tabs[1][:, :])

            out_flat = out.rearrange("t (n p) u -> (t n) p u", p=128)
            out_v = out.rearrange("t (q p) u -> t p q u", p=128)

            # tiny table-gated matmuls fill the PE wait queue (depth 4) so
            # the real matmuls' cost-model visits defer past the p-state
            # ramp threshold and run at full speed
            if pe_warm:
                wps = hpsp.tile([16, 16], FP32, tag="warm") if split0 else \
                    psp.tile([16, 16], FP32, tag="warm")
                for _ in range(pe_warm):
                    nc.tensor.matmul(
                        wps[:], tt[0][:16, :16], tt[0][:16, :16],
                        start=True, stop=True,
                    )

            res = {}
            relu_insts = {}
            # mm + relu per block; first block split in col halves with two
            # separate PSUM tiles so ScalarE and DVE relu its halves
            # concurrently (same tile would serialize the readers)
            for g in range(2):
                for q in range(4):
                    b = g * 4 + q
                    res[b] = resp.tile([128, 512], FP32, tag="res", name=f"res{b}")
                    if b == 0 and split0:
                        psa = hpsp.tile([128, 256], FP32, tag="mm0a")
                        psb = hpsp.tile([128, 256], FP32, tag="mm0b")
                        nc.tensor.matmul(
                            psa[:], tt[0][:K, :128], tt[0][:K, 512:768],
                            start=True, stop=True,
                        )
                        nc.tensor.matmul(
                            psb[:], tt[0][:K, :128], tt[0][:K, 768:1024],
                            start=True, stop=True,
                        )
                        i1 = nc.scalar.activation(
                            res[b][:, :256], psa[:],
                            mybir.ActivationFunctionType.Relu,
                        )
                        i2 = nc.vector.tensor_scalar_max(
                            res[b][:, 256:], psb[:], 0.0
                        )
                        if b in relu_sems:
                            i1.then_inc(relu_sems[b], 1)
                            i2.then_inc(relu_sems[b], 1)
                        relu_insts[b] = [i1.ins, i2.ins]
                        continue
                    ps = psp.tile([128, 512], FP32, tag="mm", name=f"mm{b}")
                    nc.tensor.matmul(
                        ps[:],
                        tt[g][:K, q * 128:(q + 1) * 128],
                        tt[g][:K, 512:1024],
                        start=True,
                        stop=True,
                    )
                    if b % 2 == 1:
                        i1 = nc.vector.tensor_scalar_max(res[b][:], ps[:], 0.0)
                    else:
                        i1 = nc.scalar.activation(
                            res[b][:], ps[:], mybir.ActivationFunctionType.Relu
                        )
                    if b in relu_sems:
                        i1.then_inc(relu_sems[b], 1)
                    relu_insts[b] = [i1.ins]

            # prepared scatter-writes for the leading blocks (zero-initialized
            # output buffer makes scatter-add a plain write); one SWDGE queue
            # each so triggers fire independently per block.  high_priority
            # makes the Tile scheduler place the preps at the top of the Pool
            # queue so descriptor gen runs during the table-DMA wait.
            from concourse.instruction_name_ordered_set import (
                InstructionNameOrderedSet,
            )

            def after(inst, prev):
                if prev is None:
                    return
                deps = InstructionNameOrderedSet()
                deps.add(prev.ins.name)
                inst.ins.add_nosync_dependencies_from(deps)

            qoff = 1 if swdge_in else 0
            preps = []
            with tc.high_priority():
                for i, b in enumerate(swdge_blocks):
                    sem_b = nc.alloc_semaphore(f"out{b}_dma")
                    # kv_writeback = prepared OVERWRITE of one [128,512] block:
                    # out view [batch=1, dhi=128, dho=1, n_ctx=512], ctx idx 0
                    p = nc.gpsimd.kv_writeback(
                        out_flat[b].rearrange("(a p) (o u) -> a p o u", a=1, o=1),
                        res[b].rearrange("p (a o u) -> p a o u", a=1, o=1),
                        zidx[:, :],
                        prepare_only=True,
                        sem=sem_b,
                        queue_num=qoff + i,
                    )
                    preps.append(p)
            # guard(wait relus) + trigger per block, pinned in block order
            # with a no-sync dependency chain (the scheduler's internal
            # model otherwise reorders them, and triggers can't carry
            # extra sync waits — codegen limit)
            prev = preps[-1] if preps else None
            for i, b in enumerate(swdge_blocks):
                guard = nc.gpsimd.wait_ge(guard_sem, 0)
                after(guard, prev)
                trig = nc.gpsimd.trigger_dma(count=None, queue_num=qoff + i)
                after(trig, guard)
                prev = trig
                trigger_waits.append((guard.ins, relu_insts[b]))

            # remaining blocks stream via HWDGE on SP
            for b in range(8):
                if b in swdge_blocks:
                    continue
                g, q = b // 4, b % 4
                nc.sync.dma_start(out_v[g, :, q], res[b][:])

    nc.finalize()
    _fix_swdge_sync(nc, trigger_waits)
    return nc


# ---------------------------------------------------------------------------
# Runner: one compiled program, dispatched concurrently onto 8 NeuronCores
# (modeled on bass2jax.run_bass_via_pjrt's single-core path).
# ---------------------------------------------------------------------------


def _make_exec(nc):
    import jax
    from concourse.bass2jax import _bass_exec_p, install_neuronx_cc_hook
    import concourse.mybir as mb

    install_neuronx_cc_hook()

    pid_name = nc.partition_id_tensor.name if nc.partition_id_tensor else None
    in_names, out_names, out_avals, zero_outs = [], [], [], []
    pid_shape_dtype = None
    for alloc in nc.m.functions[0].allocations:
        if not isinstance(alloc, mb.MemoryLocationSet):
            continue
        name = alloc.memorylocations[0].name
        if alloc.kind == "ExternalInput":
            if name == pid_name:
                pid_shape_dtype = (tuple(alloc.tensor_shape), mb.dt.np(alloc.dtype))
            in_names.append(name)
        elif alloc.kind == "ExternalOutput":
            out_names.append(name)
            shape = tuple(alloc.tensor_shape)
            dtype = mb.dt.np(alloc.dtype)
            out_avals.append(jax.core.ShapedArray(shape, dtype))
            zero_outs.append(np.zeros(shape, dtype))
    n_params = len(in_names)
    all_names = in_names + out_names

    def _body(*args):
        outs = _bass_exec_p.bind(
            *args,
            out_avals=tuple(out_avals),
            in_names=tuple(all_names),
            out_names=tuple(out_names),
            lowering_input_output_aliases=(),
            sim_require_finite=True,
            sim_require_nnan=True,
            nc=nc,
        )
        return tuple(outs)

    donate = tuple(range(n_params, n_params + len(out_names)))
    jitted = jax.jit(_body, donate_argnums=donate, keep_unused=True)
    extra = (pid_name, pid_shape_dtype) if pid_name is not None else None
    return jitted, in_names[:n_params], out_names, zero_outs, extra


_CACHE: dict = {}


def kernel(x: np.ndarray, coords: np.ndarray) -> np.ndarray:
    import time

    # transient NRT_EXEC_UNIT_UNRECOVERABLE flakes have been observed on the
    # first execution of a freshly compiled program; retry a couple of times
    last = None
    for attempt in range(3):
        try:
            return _kernel_once(x, coords)
        except Exception as e:  # jax.errors.JaxRuntimeError and friends
            last = e
            _CACHE.clear()
            time.sleep(2.0)
    raise last


def _kernel_once(x: np.ndarray, coords: np.ndarray) -> np.ndarray:
    import jax

    coords = np.asarray(coords, dtype=np.float32)
    devices = jax.devices()[:NCORES]

    entry = _CACHE.get("prog")
    if entry is None:
        nc = build_program()
        entry = _make_exec(nc)
        _CACHE["prog"] = entry
    jitted, in_names, out_names, zero_outs, extra = entry

    futures = []
    for b in range(NCORES):
        tab0, tab1 = build_tables(coords[b])
        in_map = {"tab0": tab0, "tab1": tab1}
        if extra is not None:
            in_map[extra[0]] = np.full(extra[1][0], b, dtype=extra[1][1])
        args = [jax.device_put(in_map[n], devices[b]) for n in in_names]
        args += [jax.device_put(z.copy(), devices[b]) for z in zero_outs]
        futures.append((out_names, jitted(*args)))

    outs = []
    for out_names, arrs in futures:
        res = {n: np.asarray(a) for n, a in zip(out_names, arrs)}
        outs.append(res["out"].reshape(2, H, W))
    return np.stack(outs, axis=0)
